# revision 5
# baseline (speedup 1.0000x reference)
"""Bass/Trainium2 kernel for nn_DirectedMessagePassingLayer_65807488909810.

Reference computation:
    agg_in  = segment_sum(vals_in[:,None]  * x[cols_in],  rows_in,  n)
    agg_out = segment_sum(vals_out[:,None] * x[cols_out], rows_out, n)
    h = x @ W_self.T + b_self + agg_in @ W_in.T + agg_out @ W_out.T
    out = relu(layernorm(h) * gamma + beta)

Distribution (8 NeuronCores, SPMD — one compiled program, per-core data):
  destination rows are assigned to (core, block, window) bins by a
  degree-balancing pass (we own the row permutation; the host inverse-permutes
  the output), so per-cell edge counts are nearly equal across cores and the
  shared instruction stream carries ~2% padding instead of ~20%.

Per-core algorithm:
  * Edges of BOTH adjacency sets share one slot grid: cell = (block,
    64-row window); the one-hot column is set*64 + in-window position, so a
    single PSUM tile [feat, 128] accumulates agg_in (cols 0:64) and agg_out
    (cols 64:128) for that window.
  * Slots are gathered from bf16 x tables (lo/hi split for int16 indices)
    with InstDMAGatherAnt in batches of 128*SB rows.
  * The scaled one-hot S[slot, col] = val * (col == rl) is built per stripe
    with a single fused tensor_scalar (is_equal, mult) — keeps DVE's packed
    16-bit fast path (no broadcast operands).
  * Per 128-slot chunk the tensor engine scatters: PSUM[feat, col] += G.T @ S.
  * Per block: ph = Wself.T @ xT_blk + Win.T @ agg_in + Wout.T @ agg_out in
    PSUM (bf16 weights -> fast weight load); ACT copies ph+bias to SBUF bf16;
    PE-transpose; LayerNorm stats via ACT accumulate; final
    relu((h-mu)*rstd) as one ACT op with per-partition scale/bias; bf16 store.
"""

import numpy as np
import ml_dtypes

import concourse.bass as bass
import concourse.bacc as bacc
import concourse.mybir as mybir
import concourse.tile as tile
from concourse.bass_utils import run_bass_kernel_spmd
from concourse.masks import make_identity

# ---------------- problem constants (hardcoded per contract) ----------------
N_NODES = 50000
D = 128
LN_EPS = 1e-5
N_CORES = 8
BLOCKS = 49                   # 49 blocks of 128 dest rows per core
PAD_ROWS = BLOCKS * 128       # 6272
ROWS_PER_CORE = 6250          # only used by legacy callers
WIN = 64                      # dest window width (one-hot col = set*64+pos)
CHUNK = 128
SB = 32                       # gather batch stripes (4096-row gathers)
XLO_ROWS = 32768              # lo table = x[0:XLO_ROWS]
HI_BASE = 17232               # hi table = x[HI_BASE:] (32768 rows)
NBINS = N_CORES * BLOCKS * 2  # 784 window bins, <=64 rows each

F32 = mybir.dt.float32
BF16 = mybir.dt.bfloat16
I16 = mybir.dt.int16

BF16_NP = np.dtype(ml_dtypes.bfloat16)


def _split_multi_waits(nc):
    """This walrus build encodes at most one sync-wait per instruction;
    split N-wait instructions into N-1 preceding single-wait NoOps
    (engine-serial execution preserves the semantics)."""
    k = 0
    for f in nc.m.functions:
        for bb in f.blocks:
            new = []
            for inst in bb.instructions:
                si = inst.sync_info
                if si is not None and si.on_wait is not None and len(si.on_wait) > 1:
                    waits = list(si.on_wait)
                    for w in waits[:-1]:
                        k += 1
                        new.append(mybir.InstNoOp(
                            name=f"waitsplit-{k}", engine=inst.engine,
                            ins=[], outs=[],
                            sync_info=mybir.SyncInfo(on_wait=[w], on_update=[])))
                    si.on_wait = waits[-1:]
                new.append(inst)
            bb.instructions = new
    return k


def _wrap_slots(a, nb, sb):
    """[n_slots] -> [128, stripes] with slot g at [g%128, g//128]."""
    return np.ascontiguousarray(a.reshape(nb * sb, 128).T)


def _wrap_idx16(a, nb, sb):
    """[n_slots] -> [128, stripes*16th] in dma_gather's per-batch 16-wrap:
    batch bi, in-batch j -> [j%16 (replicated x8), bi*(sb*8) + j//16]."""
    n_per_batch = sb * 128
    A = a.reshape(nb, n_per_batch // 16, 16)          # [nb, cols, 16]
    B = A.transpose(2, 0, 1).reshape(16, nb * (n_per_batch // 16))
    return np.ascontiguousarray(np.tile(B, (8, 1)))


def _balance_rows(d_in, d_out, iters=4000):
    """Assign the N_NODES dest rows to NBINS window bins (<=64 rows each) so
    per-bin total degree is near-uniform; group similar bins per (block,
    window) across the 8 cores. Returns bin_of[row] and the bin->(core,b,w)
    map as an ordering of bins."""
    t = (d_in + d_out).astype(np.int64)
    order = np.argsort(-t, kind="stable")
    bin_of = np.empty(N_NODES, dtype=np.int64)
    for wv in range(0, N_NODES, NBINS):
        ch = order[wv:wv + NBINS]
        pos = np.arange(len(ch))
        if (wv // NBINS) % 2 == 1:
            pos = NBINS - 1 - pos
        bin_of[ch] = pos
    sums = np.bincount(bin_of, weights=t, minlength=NBINS).astype(np.int64)
    rows_by_bin = [[] for _ in range(NBINS)]
    for r in range(N_NODES):
        rows_by_bin[bin_of[r]].append(r)

    # pairwise swap refinement: repeatedly swap one row between the heaviest
    # and lightest bins to move their sums toward each other.
    for _ in range(iters):
        hi = int(np.argmax(sums))
        lo = int(np.argmin(sums))
        gap = sums[hi] - sums[lo]
        if gap <= 2:
            break
        rh = np.asarray(rows_by_bin[hi])
        rl = np.asarray(rows_by_bin[lo])
        # best single-row swap: delta = t[rh] - t[rl], want delta ~ gap/2
        th, tl = t[rh], t[rl]
        dmat_best, best = None, None
        # vector search: for each rh row, best rl row is the one minimizing
        # |gap/2 - (th - tl)| -> search via sorted tl
        sl = np.sort(tl)
        want = th.astype(np.float64) - gap / 2.0
        ins = np.clip(np.searchsorted(sl, want), 0, len(sl) - 1)
        cand = np.abs(sl[ins] - want)
        i = int(np.argmin(cand))
        delta = float(th[i]) - float(sl[ins[i]])
        if delta <= 0:
            break
        j = int(np.flatnonzero(tl == sl[ins[i]])[0])
        a, b = int(rh[i]), int(rl[j])
        rows_by_bin[hi].remove(a); rows_by_bin[hi].append(b)
        rows_by_bin[lo].remove(b); rows_by_bin[lo].append(a)
        bin_of[a], bin_of[b] = lo, hi
        sums[hi] += t[b] - t[a]
        sums[lo] += t[a] - t[b]
    # order bins by sum desc; consecutive groups of 8 share a (b, w) cell
    bin_order = np.argsort(-sums, kind="stable")
    return bin_of, bin_order


def _build_layout(x_cols_ok, edge_sets, d_in, d_out):
    """Returns layout dict: prog, nb, wrapped per-core arrays, node map."""
    bin_of, bin_order = _balance_rows(d_in, d_out)
    # bin -> (core, b, w): group g = bin_order[8g:8g+8] -> cores 0..7;
    # g -> (b, w) = (g // 2, g % 2)
    core_of_bin = np.empty(NBINS, dtype=np.int64)
    b_of_bin = np.empty(NBINS, dtype=np.int64)
    w_of_bin = np.empty(NBINS, dtype=np.int64)
    for g in range(NBINS // N_CORES):
        bins = bin_order[g * N_CORES:(g + 1) * N_CORES]
        core_of_bin[bins] = np.arange(N_CORES)
        b_of_bin[bins] = g // 2
        w_of_bin[bins] = g % 2

    # rows -> (core, b, w, pos) and node_of map
    core_of = core_of_bin[bin_of]
    b_of = b_of_bin[bin_of]
    w_of = w_of_bin[bin_of]
    pos_of = np.zeros(N_NODES, dtype=np.int64)
    node_of = np.full((N_CORES, PAD_ROWS), -1, dtype=np.int64)
    binkey = bin_of
    order = np.argsort(binkey, kind="stable")
    counts = np.bincount(binkey, minlength=NBINS)
    starts = np.concatenate([[0], np.cumsum(counts)[:-1]])
    pos_of[order] = np.arange(N_NODES) - starts[binkey[order]]
    assert pos_of.max() < WIN
    p_of = b_of * 128 + w_of * WIN + pos_of
    node_of[core_of, p_of] = np.arange(N_NODES)

    # ---- edges: merged sets, keyed by (core, b, w) ----
    r = np.concatenate([edge_sets[0][0], edge_sets[1][0]])
    c = np.concatenate([edge_sets[0][1], edge_sets[1][1]])
    v = np.concatenate([edge_sets[0][2], edge_sets[1][2]])
    sset = np.concatenate([np.zeros(len(edge_sets[0][0]), np.int64),
                           np.ones(len(edge_sets[1][0]), np.int64)])
    ecore = core_of[r]
    eb = b_of[r]
    ew = w_of[r]
    ecol = sset * WIN + pos_of[r]                    # one-hot column 0..127
    ncell = BLOCKS * 2
    key = (ecore * ncell + eb * 2 + ew).astype(np.int64)

    hclass = np.ones(len(c), dtype=np.int64)         # 1 = flex
    hclass[c < HI_BASE] = 0                          # must-lo
    hclass[c >= XLO_ROWS] = 2                        # must-hi

    nk = N_CORES * ncell
    n_all = np.bincount(key, minlength=nk).reshape(N_CORES, ncell)
    n_mlo = np.bincount(key[hclass == 0], minlength=nk).reshape(N_CORES, ncell)
    n_mhi = np.bincount(key[hclass == 2], minlength=nk).reshape(N_CORES, ncell)
    n_flex = n_all - n_mlo - n_mhi

    T = np.maximum(1, -(-n_all.max(axis=0) // CHUNK))        # [ncell]
    Lmin = -(-n_mlo.max(axis=0) // CHUNK)
    Hmin = -(-n_mhi.max(axis=0) // CHUNK)
    T = np.maximum(T, Lmin + Hmin)
    # lo share ~ (must_lo + flex/2) / n
    tot = np.maximum(1, n_all.sum(axis=0))
    frac = (n_mlo.sum(axis=0) + n_flex.sum(axis=0) / 2.0) / tot
    L = np.clip(np.round(T * frac).astype(np.int64), Lmin, T - Hmin)
    H = T - L

    # per-core lo counts
    lo_c = np.clip(n_all - H[None, :] * CHUNK, n_mlo, n_mlo + n_flex)
    assert (lo_c <= L[None, :] * CHUNK).all()
    assert (n_all - lo_c <= H[None, :] * CHUNK).all()

    # stream bases per cell (program order: b asc, w asc), per stream
    cells = np.arange(ncell)
    base = [np.zeros(ncell, np.int64), np.zeros(ncell, np.int64)]
    p0 = p1 = 0
    for cell in cells:
        base[0][cell] = p0
        base[1][cell] = p1
        p0 += int(L[cell]) * CHUNK
        p1 += int(H[cell]) * CHUNK
    n_slots = [p0, p1]
    batch = CHUNK * SB
    nb = [max(1, -(-n // batch)) for n in n_slots]

    # edge stream assignment: rank of flex edges within (key) decides lo/hi
    stream = np.zeros(len(c), dtype=np.int64)
    stream[hclass == 2] = 1
    fi = np.flatnonzero(hclass == 1)
    fo = fi[np.argsort(key[fi], kind="stable")]
    f_cnt = np.bincount(key[fi], minlength=nk)
    f_start = np.concatenate([[0], np.cumsum(f_cnt)[:-1]])
    frank = np.arange(len(fo)) - f_start[key[fo]]
    quota = (lo_c - n_mlo).reshape(-1)[key[fo]]
    stream[fo] = (frank >= quota).astype(np.int64)

    # slot index within (key, stream): stable rank
    skey = key * 2 + stream
    so = np.argsort(skey, kind="stable")
    s_cnt = np.bincount(skey, minlength=nk * 2)
    s_start = np.concatenate([[0], np.cumsum(s_cnt)[:-1]])
    srank = np.arange(len(so)) - s_start[skey[so]]

    idx_a = [np.zeros((N_CORES, nb[h] * batch), dtype=np.int16) for h in range(2)]
    rl_a = [np.zeros((N_CORES, nb[h] * batch), dtype=np.float32) for h in range(2)]
    val_a = [np.zeros((N_CORES, nb[h] * batch), dtype=np.float32) for h in range(2)]
    ecell = key[so] % ncell
    ecore_o = key[so] // ncell
    h_o = stream[so]
    slotpos = np.where(h_o == 0, base[0][ecell], base[1][ecell]) + srank
    for h in range(2):
        m = h_o == h
        idx_a[h][ecore_o[m], slotpos[m]] = (c[so][m] - h * HI_BASE).astype(np.int16)
        rl_a[h][ecore_o[m], slotpos[m]] = ecol[so][m]
        val_a[h][ecore_o[m], slotpos[m]] = v[so][m]

    prog = []
    for b in range(BLOCKS):
        prog.append([(wv, int(L[b * 2 + wv]), int(H[b * 2 + wv]))
                     for wv in range(2)])

    out = {"prog": prog, "nb": nb, "node_of": node_of,
           "n_slots": n_slots}
    for h in range(2):
        out[f"idx{h}"] = np.stack([_wrap_idx16(idx_a[h][ci], nb[h], SB)
                                   for ci in range(N_CORES)])
        out[f"rl{h}"] = np.stack([_wrap_slots(rl_a[h][ci], nb[h], SB)
                                  for ci in range(N_CORES)])
        out[f"val{h}"] = np.stack([_wrap_slots(val_a[h][ci], nb[h], SB)
                                   for ci in range(N_CORES)])
    return out


def _trace_kernel(nc, prog, nb, gamma_trivial, beta_trivial, repeats=1):
    stripes = [nb[h] * SB for h in range(2)]
    icolumns = [nb[h] * SB * 8 for h in range(2)]     # int16 idx columns

    xlo = nc.declare_dram_parameter("xlo", [XLO_ROWS, D], BF16, isOutput=False)
    xhi = nc.declare_dram_parameter("xhi", [N_NODES - HI_BASE, D], BF16,
                                    isOutput=False)
    xT = nc.declare_dram_parameter("xT", [D, PAD_ROWS], BF16, isOutput=False)
    WselfT = nc.declare_dram_parameter("WselfT", [D, D], BF16, isOutput=False)
    WinT = nc.declare_dram_parameter("WinT", [D, D], BF16, isOutput=False)
    WoutT = nc.declare_dram_parameter("WoutT", [D, D], BF16, isOutput=False)
    bself = nc.declare_dram_parameter("bself", [D, 1], F32, isOutput=False)
    iota_d = nc.declare_dram_parameter("iota", [128, 128], BF16, isOutput=False)
    idx_d, rl_d, val_d = [], [], []
    for h in range(2):
        idx_d.append(nc.declare_dram_parameter(f"idx{h}", [128, icolumns[h]],
                                               I16, isOutput=False))
        rl_d.append(nc.declare_dram_parameter(f"rl{h}", [128, stripes[h]],
                                              F32, isOutput=False))
        val_d.append(nc.declare_dram_parameter(f"val{h}", [128, stripes[h]],
                                               F32, isOutput=False))
    if not gamma_trivial:
        gamma_d = nc.declare_dram_parameter("gamma_rep", [128, D], F32,
                                            isOutput=False)
    if not beta_trivial:
        beta_d = nc.declare_dram_parameter("beta_rep", [128, D], F32,
                                           isOutput=False)
    out_d = nc.declare_dram_parameter("out", [PAD_ROWS, D], BF16, isOutput=True)

    xtab = [xlo, xhi]

    with tile.TileContext(nc) as tc:
        with (
            tc.tile_pool(name="const", bufs=1) as constp,
            tc.tile_pool(name="g0", bufs=2) as g0pool,
            tc.tile_pool(name="g1", bufs=2) as g1pool,
            tc.tile_pool(name="meta", bufs=4) as mpool,
            tc.tile_pool(name="sbuf", bufs=3) as spool,
            tc.tile_pool(name="outp", bufs=4) as opool,
            tc.tile_pool(name="psumA", bufs=4, space="PSUM") as psA,
            tc.tile_pool(name="psumH", bufs=2, space="PSUM") as psH,
        ):
            gpool = [g0pool, g1pool]
            # ---- constants ----
            WselfT_s = constp.tile([D, D], BF16, tag="wself")
            WinT_s = constp.tile([D, D], BF16, tag="win")
            WoutT_s = constp.tile([D, D], BF16, tag="wout")
            bself_s = constp.tile([D, 1], F32, tag="bself")
            identB = constp.tile([128, 128], BF16, tag="ident")
            xT_s = constp.tile([D, PAD_ROWS], BF16, tag="xt")
            iota_s = constp.tile([128, 128], BF16, tag="iota")
            scratch = constp.tile([128, 128], BF16, tag="scratch")
            nc.sync.dma_start(out=WselfT_s[:], in_=WselfT[:])
            nc.sync.dma_start(out=WinT_s[:], in_=WinT[:])
            nc.sync.dma_start(out=WoutT_s[:], in_=WoutT[:])
            nc.sync.dma_start(out=bself_s[:], in_=bself[:])
            nc.sync.dma_start(out=xT_s[:], in_=xT[:])
            nc.sync.dma_start(out=iota_s[:], in_=iota_d[:])
            make_identity(nc, identB[:])
            if not gamma_trivial:
                gamma_s = constp.tile([128, D], F32, tag="gamma")
                nc.sync.dma_start(out=gamma_s[:], in_=gamma_d[:])
            if not beta_trivial:
                beta_s = constp.tile([128, D], F32, tag="beta")
                nc.sync.dma_start(out=beta_s[:], in_=beta_d[:])

            for rep in range(repeats):
                # ---- per-stream gather batches ----
                state = [{"batch": None, "cursor": 0},
                         {"batch": None, "cursor": 0}]

                def make_batch(h, bi, rep=rep):
                    gt = gpool[h].tile([128, SB, D], BF16, tag="g")
                    it = mpool.tile([128, SB * 8], I16, tag=f"idx{h}")
                    rt = mpool.tile([128, SB], F32, tag=f"rl{h}")
                    vt = mpool.tile([128, SB], F32, tag=f"val{h}")
                    St = gpool[h].tile([128, SB, 128], BF16, tag="s")
                    c0 = bi * SB * 8
                    nc.sync.dma_start(out=it[:], in_=idx_d[h][:, c0:c0 + SB * 8])
                    nc.sync.dma_start(out=rt[:],
                                      in_=rl_d[h][:, bi * SB:(bi + 1) * SB])
                    nc.sync.dma_start(out=vt[:],
                                      in_=val_d[h][:, bi * SB:(bi + 1) * SB])
                    nc.gpsimd.dma_gather(
                        out_ap=gt[:], in_ap=xtab[h][:], idxs_ap=it[:],
                        num_idxs=SB * 128, num_idxs_reg=SB * 128, elem_size=D,
                        single_packet=False)
                    for j in range(SB):
                        nc.vector.tensor_scalar(
                            out=St[:, j, :], in0=iota_s[:],
                            scalar1=rt[:, j:j + 1], scalar2=vt[:, j:j + 1],
                            op0=mybir.AluOpType.is_equal,
                            op1=mybir.AluOpType.mult)
                    return gt, St

                def chunk_tiles(h):
                    st = state[h]
                    bi, off = divmod(st["cursor"], SB)
                    if off == 0:
                        st["batch"] = make_batch(h, bi)
                    st["cursor"] += 1
                    gt, St = st["batch"]
                    return gt[:, off, :], St[:, off, :]

                for b in range(BLOCKS):
                    # aggC[:, 0:128] = agg_in, aggC[:, 128:256] = agg_out
                    aggC = spool.tile([128, 2, 128], BF16, tag="aggc",
                                      name=f"aggc_r{rep}_b{b}")
                    for (wv, c_lo, c_hi) in prog[b]:
                        pa = psA.tile([128, 2, WIN], F32, tag="pa",
                                      space="PSUM",
                                      name=f"pa_r{rep}_b{b}_w{wv}")
                        total = c_lo + c_hi
                        k = 0
                        for h, cnt in ((0, c_lo), (1, c_hi)):
                            for _ in range(cnt):
                                g_ap, s_ap = chunk_tiles(h)
                                nc.tensor.matmul(
                                    out=pa[:], lhsT=g_ap, rhs=s_ap,
                                    start=(k == 0), stop=(k == total - 1))
                                k += 1
                        # one copy per window: [set, 64] -> strided slices of
                        # aggC at [set, wv*WIN]
                        nc.scalar.activation(
                            out=aggC[:, :, wv * WIN:(wv + 1) * WIN],
                            in_=pa[:],
                            func=mybir.ActivationFunctionType.Copy)

                    ph = psH.tile([128, 128], F32, tag="ph", space="PSUM")
                    nc.tensor.matmul(out=ph[:], lhsT=WselfT_s[:],
                                     rhs=xT_s[:, b * 128:(b + 1) * 128],
                                     start=True, stop=False)
                    nc.tensor.matmul(out=ph[:], lhsT=WinT_s[:],
                                     rhs=aggC[:, 0, :],
                                     start=False, stop=False)
                    nc.tensor.matmul(out=ph[:], lhsT=WoutT_s[:],
                                     rhs=aggC[:, 1, :],
                                     start=False, stop=True)
                    hT = spool.tile([128, 128], BF16, tag="ht")
                    nc.scalar.activation(out=hT[:], in_=ph[:],
                                         func=mybir.ActivationFunctionType.Identity,
                                         bias=bself_s[:, :1])
                    pt = psH.tile([128, 128], BF16, tag="pt", space="PSUM")
                    nc.tensor.transpose(out=pt[:], in_=hT[:], identity=identB[:])

                    # layernorm over free dim + relu
                    hS = spool.tile([128, 128], BF16, tag="hs")
                    ssum = spool.tile([128, 1], F32, tag="ssum")
                    nc.scalar.activation(out=hS[:], in_=pt[:],
                                         func=mybir.ActivationFunctionType.Copy,
                                         accum_out=ssum[:])
                    sqsum = spool.tile([128, 1], F32, tag="sqsum")
                    nc.scalar.activation(out=scratch[:], in_=hS[:],
                                         func=mybir.ActivationFunctionType.Square,
                                         accum_out=sqsum[:])
                    mu = spool.tile([128, 1], F32, tag="mu")
                    nc.vector.tensor_scalar_mul(out=mu[:], in0=ssum[:],
                                                scalar1=1.0 / D)
                    t2 = spool.tile([128, 1], F32, tag="t2")
                    nc.vector.tensor_scalar(out=t2[:], in0=sqsum[:],
                                            scalar1=1.0 / D, scalar2=LN_EPS,
                                            op0=mybir.AluOpType.mult,
                                            op1=mybir.AluOpType.add)
                    musq = spool.tile([128, 1], F32, tag="musq")
                    nc.vector.tensor_tensor(out=musq[:], in0=mu[:], in1=mu[:],
                                            op=mybir.AluOpType.mult)
                    var = spool.tile([128, 1], F32, tag="var")
                    nc.vector.tensor_tensor(out=var[:], in0=t2[:], in1=musq[:],
                                            op=mybir.AluOpType.subtract)
                    sd = spool.tile([128, 1], F32, tag="sd")
                    nc.scalar.activation(out=sd[:], in_=var[:],
                                         func=mybir.ActivationFunctionType.Sqrt)
                    rstd = spool.tile([128, 1], F32, tag="rstd")
                    nc.vector.reciprocal(out=rstd[:], in_=sd[:])
                    nbias = spool.tile([128, 1], F32, tag="nbias")
                    nc.vector.tensor_tensor(out=nbias[:], in0=mu[:],
                                            in1=rstd[:],
                                            op=mybir.AluOpType.mult)
                    nc.vector.tensor_scalar_mul(out=nbias[:], in0=nbias[:],
                                                scalar1=-1.0)
                    ot = opool.tile([128, 128], BF16, tag="o")
                    if gamma_trivial and beta_trivial:
                        nc.scalar.activation(
                            out=ot[:], in_=hS[:],
                            func=mybir.ActivationFunctionType.Relu,
                            scale=rstd[:, :1], bias=nbias[:, :1])
                    else:
                        nrm = opool.tile([128, 128], F32, tag="nrm")
                        nc.vector.tensor_scalar(out=nrm[:], in0=hS[:],
                                                scalar1=rstd[:, :1],
                                                scalar2=nbias[:, :1],
                                                op0=mybir.AluOpType.mult,
                                                op1=mybir.AluOpType.add)
                        if not gamma_trivial:
                            nc.vector.tensor_tensor(out=nrm[:], in0=nrm[:],
                                                    in1=gamma_s[:],
                                                    op=mybir.AluOpType.mult)
                        if not beta_trivial:
                            nc.vector.tensor_tensor(out=nrm[:], in0=nrm[:],
                                                    in1=beta_s[:],
                                                    op=mybir.AluOpType.add)
                        nc.scalar.activation(
                            out=ot[:], in_=nrm[:],
                            func=mybir.ActivationFunctionType.Relu)
                    nc.sync.dma_start(out=out_d[b * 128:(b + 1) * 128, :],
                                      in_=ot[:])
                assert state[0]["cursor"] <= nb[0] * SB
                assert state[1]["cursor"] <= nb[1] * SB


def build(x, adj_in_rows, adj_in_cols, adj_in_vals,
          adj_out_rows, adj_out_cols, adj_out_vals,
          W_self, b_self, W_in, W_out, ln_gamma, ln_beta, repeats=1):
    """Trace + compile; returns (nc, in_maps, layout)."""
    x = np.asarray(x, dtype=np.float32)
    sets = [
        (np.asarray(adj_in_rows, np.int64), np.asarray(adj_in_cols, np.int64),
         np.asarray(adj_in_vals, np.float32)),
        (np.asarray(adj_out_rows, np.int64), np.asarray(adj_out_cols, np.int64),
         np.asarray(adj_out_vals, np.float32)),
    ]
    W_self = np.asarray(W_self, np.float32)
    W_in = np.asarray(W_in, np.float32)
    W_out = np.asarray(W_out, np.float32)
    b_self = np.asarray(b_self, np.float32)
    ln_gamma = np.asarray(ln_gamma, np.float32)
    ln_beta = np.asarray(ln_beta, np.float32)

    d_in = np.bincount(sets[0][0], minlength=N_NODES).astype(np.int64)
    d_out = np.bincount(sets[1][0], minlength=N_NODES).astype(np.int64)
    lay = _build_layout(None, sets, d_in, d_out)
    gamma_trivial = bool(np.all(ln_gamma == 1.0))
    beta_trivial = bool(np.all(ln_beta == 0.0))

    nc = bacc.Bacc("TRN2", target_bir_lowering=False, debug=False,
                   num_devices=N_CORES, dynamic_dma_scratch_size=81920)
    _trace_kernel(nc, lay["prog"], lay["nb"], gamma_trivial, beta_trivial,
                  repeats=repeats)
    nc.compile()

    xlo = np.ascontiguousarray(x[:XLO_ROWS]).astype(BF16_NP)
    xhi = np.ascontiguousarray(x[HI_BASE:]).astype(BF16_NP)
    iota = np.tile(np.arange(128, dtype=np.float32)[None, :],
                   (128, 1)).astype(BF16_NP)
    node_of = lay["node_of"]
    in_maps = []
    for ci in range(N_CORES):
        xT_c = np.zeros((D, PAD_ROWS), dtype=np.float32)
        valid = node_of[ci] >= 0
        xT_c[:, valid] = x[node_of[ci][valid]].T
        m = {
            "xlo": xlo, "xhi": xhi, "xT": xT_c.astype(BF16_NP),
            "WselfT": np.ascontiguousarray(W_self.T).astype(BF16_NP),
            "WinT": np.ascontiguousarray(W_in.T).astype(BF16_NP),
            "WoutT": np.ascontiguousarray(W_out.T).astype(BF16_NP),
            "bself": np.ascontiguousarray(b_self[:, None]),
            "iota": iota,
        }
        for h in range(2):
            m[f"idx{h}"] = lay[f"idx{h}"][ci]
            m[f"rl{h}"] = lay[f"rl{h}"][ci]
            m[f"val{h}"] = lay[f"val{h}"][ci]
        if not gamma_trivial:
            m["gamma_rep"] = np.tile(ln_gamma[None, :], (128, 1))
        if not beta_trivial:
            m["beta_rep"] = np.tile(ln_beta[None, :], (128, 1))
        in_maps.append(m)
    return nc, in_maps, lay


_LAST_LAYOUT = None


def assemble_output(res):
    node_of = _LAST_LAYOUT["node_of"]
    full = np.zeros((N_NODES, D), dtype=np.float32)
    for ci in range(N_CORES):
        o = np.asarray(res[ci]["out"]).astype(np.float32)
        valid = node_of[ci] >= 0
        full[node_of[ci][valid]] = o[valid]
    return full


def kernel(**inputs):
    global _LAST_LAYOUT
    nc, in_maps, lay = build(**inputs)
    _LAST_LAYOUT = lay
    _split_multi_waits(nc)
    res = run_bass_kernel_spmd(nc, in_maps, core_ids=list(range(N_CORES)))
    return assemble_output(res.results)


def make_timed_runner(nc, in_maps, n_cores):
    """Jitted 8-core SPMD executable with repeat-callable timing (mirrors
    concourse.bass2jax.run_bass_via_pjrt's multi-core path)."""
    import time
    import jax
    from jax.experimental.shard_map import shard_map
    from jax.sharding import Mesh, PartitionSpec, NamedSharding
    from concourse.bass2jax import _bass_exec_p, install_neuronx_cc_hook, \
        partition_id_tensor

    install_neuronx_cc_hook()
    partition_name = nc.partition_id_tensor.name if nc.partition_id_tensor else None
    in_names, out_names, out_avals, zero_outs = [], [], [], []
    for alloc in nc.m.functions[0].allocations:
        if not isinstance(alloc, mybir.MemoryLocationSet):
            continue
        name = alloc.memorylocations[0].name
        if alloc.kind == "ExternalInput":
            if name != partition_name:
                in_names.append(name)
        elif alloc.kind == "ExternalOutput":
            shape = tuple(alloc.tensor_shape)
            dtype = mybir.dt.np(alloc.dtype)
            out_names.append(name)
            out_avals.append(jax.core.ShapedArray(shape, dtype))
            zero_outs.append(np.zeros(shape, dtype))
    n_params, n_outs = len(in_names), len(out_avals)
    all_in_names = list(in_names) + list(out_names)
    if partition_name is not None:
        all_in_names.append(partition_name)

    def _body(*args):
        operands = list(args)
        if partition_name is not None:
            operands.append(partition_id_tensor())
        return tuple(_bass_exec_p.bind(
            *operands, out_avals=tuple(out_avals), in_names=tuple(all_in_names),
            out_names=tuple(out_names), lowering_input_output_aliases=(),
            sim_require_finite=True, sim_require_nnan=True, nc=nc))

    devices = jax.devices()[:n_cores]
    mesh = Mesh(np.asarray(devices), ("core",))
    in_specs = (PartitionSpec("core"),) * (n_params + n_outs)
    out_specs = (PartitionSpec("core"),) * n_outs
    sharded = jax.jit(
        shard_map(_body, mesh=mesh, in_specs=in_specs, out_specs=out_specs,
                  check_rep=False),
        donate_argnums=tuple(range(n_params, n_params + n_outs)),
        keep_unused=True)
    shard0 = NamedSharding(mesh, PartitionSpec("core"))
    dev_in = [jax.device_put(
        np.concatenate([np.asarray(in_maps[c][nm]) for c in range(n_cores)], axis=0),
        shard0) for nm in in_names]
    concat_zeros = [np.zeros((n_cores * z.shape[0], *z.shape[1:]), z.dtype)
                    for z in zero_outs]

    def run():
        dev_zeros = [jax.device_put(a, shard0) for a in concat_zeros]
        jax.block_until_ready(dev_zeros)
        t0 = time.perf_counter()
        outs = sharded(*dev_in, *dev_zeros)
        jax.block_until_ready(outs)
        return outs, time.perf_counter() - t0

    def results(outs):
        res = []
        for c in range(n_cores):
            d = {}
            for i, nm in enumerate(out_names):
                per = np.asarray(outs[i])
                rows = per.shape[0] // n_cores
                d[nm] = per[c * rows:(c + 1) * rows]
            res.append(d)
        return res

    return run, results
